# revision 5
# baseline (speedup 1.0000x reference)
"""AttentionBlock kernel for 8 Trainium2 NeuronCores.

Problem (hardcoded): x [4, 2048, 1024] f32; Wq/Wk/Wv/Wfc [1024, 1024]; biases [1024].
    q = x@Wq.T+bq; k = x@Wk.T+bk; v = x@Wv.T+bv
    out = softmax(q k^T / sqrt(1024)) v;  y = out@Wfc.T+bfc + x

Sharding: core i = (b = i//2, h = i%2). Each core computes the full K/V for its
batch element (duplicated across the 2 cores sharing a batch) and the attention +
fc for its half of the sequence. No collectives needed.

Per-core layout strategy (all matmuls in float32r = full PE rate):
  host feeds xT = x[b].T (d-major) and pre-transposed weights (d-major), so every
  GEMM has its contraction dim on partitions with no on-device transposes.
  - K^T [e, s] tiles -> spilled to DRAM, streamed back per attention q-chunk
  - V   [s, e] resident in SBUF; Q^T [e, q] resident
  - S^T = K^T.T-blocks @ Q^T; softmax over the partition (k) axis:
    exp without max-subtract (|S| <~ 6 here), denominator via ones-matmul,
    reciprocal broadcast across partitions with a rank-1 PE matmul.
  - U^T = V-blocks.T @ expS^T accumulated in PSUM; normalized on copy-out
  - y = (O^T-blocks).T @ Wfc^T + bfc + x  (row-scaling by 1/denom folded in copyout)
"""

import numpy as np

B, S, DIM = 4, 2048, 1024
P = 128
NCORES = 8
HALF = S // 2          # 1024 q rows per core
DT = DIM // P          # 8 d tiles
ET = DIM // P          # 8 e tiles
SCH = S // 512         # 4 s-chunks for K proj
ST = S // P            # 16 s tiles for V proj
QC = 256               # attention q-chunk
NQ = HALF // QC        # 4 q chunks
KB = S // P            # 16 k blocks
SCALE = 1.0 / float(np.sqrt(DIM))

_CACHE = {}


def _build():
    import concourse.mybir as mybir
    import concourse.tile as tile
    from concourse import bacc

    F32 = mybir.dt.float32
    F32R = mybir.dt.float32r
    EXP = mybir.ActivationFunctionType.Exp
    IDENT = mybir.ActivationFunctionType.Identity
    ADD = mybir.AluOpType.add
    MULT = mybir.AluOpType.mult

    nc = bacc.Bacc()

    xt_d = nc.dram_tensor("xt", [DIM, S], F32R, kind="ExternalInput")
    xr_d = nc.dram_tensor("xr", [HALF, DIM], F32, kind="ExternalInput")
    wq_d = nc.dram_tensor("wq", [DIM, DIM], F32R, kind="ExternalInput")
    wk_d = nc.dram_tensor("wk", [DIM, DIM], F32R, kind="ExternalInput")
    wv_d = nc.dram_tensor("wv", [DIM, DIM], F32R, kind="ExternalInput")
    wf_d = nc.dram_tensor("wf", [DIM, DIM], F32R, kind="ExternalInput")
    bq_d = nc.dram_tensor("bq", [DIM], F32, kind="ExternalInput")
    bk_d = nc.dram_tensor("bk", [DIM], F32, kind="ExternalInput")
    bv_d = nc.dram_tensor("bv", [DIM], F32, kind="ExternalInput")
    bf_d = nc.dram_tensor("bf", [DIM], F32, kind="ExternalInput")
    y_d = nc.dram_tensor("y", [HALF, DIM], F32, kind="ExternalOutput")
    kt_d = nc.dram_tensor("ktspill", [ET, P, S], F32R)  # K^T spill [et, e_p, s]

    xt3 = xt_d[:].rearrange("(dt p) s -> p dt s", p=P)      # [128, 8, 2048]
    ktd3 = kt_d[:].rearrange("et p s -> p et s")            # [128, 8, 2048]

    with tile.TileContext(nc, pool_alloc_mode="queue") as tc:
        cpool = tc.alloc_tile_pool(name="const", bufs=1)
        bqc = cpool.tile([P, ET], F32)   # bq as columns per e-tile
        bkc = cpool.tile([P, ET], F32)
        bvb = cpool.tile([P, DIM], F32)  # bv broadcast over partitions
        bfb = cpool.tile([P, DIM], F32)
        onesk = cpool.tile([P, 1], F32R)   # denominator lhsT
        ones1 = cpool.tile([1, P], F32R)   # partition-broadcast lhsT
        ones_f32 = cpool.tile([P, P], F32)
        nc.sync.dma_start(bqc[:], bq_d[:].rearrange("(t p) -> p t", p=P))
        nc.sync.dma_start(bkc[:], bk_d[:].rearrange("(t p) -> p t", p=P))
        nc.sync.dma_start(bvb[:], bv_d[:][None, :].to_broadcast((P, DIM)))
        nc.sync.dma_start(bfb[:], bf_d[:][None, :].to_broadcast((P, DIM)))
        nc.vector.memset(ones_f32[:], 1.0)
        nc.vector.tensor_copy(onesk[:], ones_f32[:, 0:1])
        nc.vector.tensor_copy(ones1[:], ones_f32[0:1, :])

        # ---------------- Phase K: K^T projection -> DRAM spill ----------------
        with tc.tile_pool(name="wk", bufs=1) as wkp, \
             tc.tile_pool(name="xtk", bufs=2) as xtkp, \
             tc.tile_pool(name="ktc", bufs=3) as ktcp, \
             tc.tile_pool(name="pk", bufs=3, space="PSUM") as pkp:
            wk_sb = wkp.tile([P, DT, DIM], F32R)
            nc.sync.dma_start(wk_sb[:], wk_d[:].rearrange("(dt p) e -> p dt e", p=P))
            for sch in range(SCH):
                xtk = xtkp.tile([P, DT, 512], F32R, tag="xtk")
                nc.sync.dma_start(xtk[:], xt3[:, :, sch * 512:(sch + 1) * 512])
                for et in range(ET):
                    ps = pkp.tile([P, 512], F32, tag="pk")
                    for dt in range(DT):
                        nc.tensor.matmul(
                            ps[:], wk_sb[:, dt, et * P:(et + 1) * P], xtk[:, dt, :],
                            start=(dt == 0), stop=(dt == DT - 1))
                    ktc = ktcp.tile([P, 512], F32R, tag="ktc")
                    nc.scalar.activation(ktc[:], ps[:], IDENT, bias=bkc[:, et:et + 1])
                    nc.sync.dma_start(kt_d[et, :, sch * 512:(sch + 1) * 512], ktc[:])

        # ---------------- Phase V: V projection (resident) ----------------
        vpool = tc.alloc_tile_pool(name="v", bufs=1)
        v_sb = vpool.tile([P, ST, DIM], F32R)  # [s_p, s_tile, e]
        with tc.tile_pool(name="wv", bufs=1) as wvp, \
             tc.tile_pool(name="xtv", bufs=3) as xtvp, \
             tc.tile_pool(name="pv", bufs=3, space="PSUM") as pvp:
            wv_sb = wvp.tile([P, DT, DIM], F32R)
            nc.sync.dma_start(wv_sb[:], wv_d[:].rearrange("(dt p) e -> p dt e", p=P))
            for st in range(ST):
                xtv = xtvp.tile([P, DT, P], F32R, tag="xtv")
                nc.sync.dma_start(xtv[:], xt3[:, :, st * P:(st + 1) * P])
                for eh in range(2):
                    ps = pvp.tile([P, 512], F32, tag="pv")
                    for dt in range(DT):
                        nc.tensor.matmul(
                            ps[:], xtv[:, dt, :], wv_sb[:, dt, eh * 512:(eh + 1) * 512],
                            start=(dt == 0), stop=(dt == DT - 1))
                    nc.vector.tensor_tensor(
                        v_sb[:, st, eh * 512:(eh + 1) * 512], ps[:],
                        bvb[:, eh * 512:(eh + 1) * 512], ADD)

        # ---------------- Phase Q: Q^T projection (resident) ----------------
        qpool = tc.alloc_tile_pool(name="qt", bufs=1)
        qt_sb = qpool.tile([P, ET, HALF], F32R)  # [e_p, e_tile, q]
        with tc.tile_pool(name="wq", bufs=1) as wqp, \
             tc.tile_pool(name="xtq", bufs=1) as xtqp, \
             tc.tile_pool(name="pq", bufs=3, space="PSUM") as pqp:
            wq_sb = wqp.tile([P, DT, DIM], F32R)
            nc.sync.dma_start(wq_sb[:], wq_d[:].rearrange("(dt p) e -> p dt e", p=P))
            for qch in range(HALF // 512):
                xtq = xtqp.tile([P, DT, 512], F32R, tag="xtq")
                # this core's q rows sit at columns h*HALF + ... of xT; the host
                # passes xq_off via closure below (baked per build) -- instead we
                # bake h into the DMA source slice using the fact that cores of
                # h=0/1 receive different xr/y anyway. To keep one SPMD program
                # for all cores, the host passes xT already rolled so that this
                # core's q half is ALWAYS columns [0, HALF).
                nc.sync.dma_start(xtq[:], xt3[:, :, qch * 512:(qch + 1) * 512])
                for et in range(ET):
                    ps = pqp.tile([P, 512], F32, tag="pq")
                    for dt in range(DT):
                        nc.tensor.matmul(
                            ps[:], wq_sb[:, dt, et * P:(et + 1) * P], xtq[:, dt, :],
                            start=(dt == 0), stop=(dt == DT - 1))
                    nc.scalar.activation(
                        qt_sb[:, et, qch * 512:(qch + 1) * 512], ps[:], IDENT,
                        bias=bqc[:, et:et + 1])

        # ---------------- Phase A: attention (per q-chunk) ----------------
        opool = tc.alloc_tile_pool(name="ot", bufs=1)
        ot_sb = opool.tile([P, ET, HALF], F32R)  # O^T (normalized) [e_p, e_tile, q]
        with tc.tile_pool(name="es", bufs=1) as esp, \
             tc.tile_pool(name="ktb", bufs=3) as ktbp, \
             tc.tile_pool(name="rec", bufs=2) as recp, \
             tc.tile_pool(name="ps_s", bufs=2, space="PSUM") as psp, \
             tc.tile_pool(name="ps_u", bufs=2, space="PSUM") as pup, \
             tc.tile_pool(name="ps_d", bufs=2, space="PSUM") as pdp:
            for qc in range(NQ):
                q0 = qc * QC
                es = esp.tile([P, KB, QC], F32R, tag="es")  # exp(S^T) [k_p, kb, q]
                for kb in range(KB):
                    ktb = ktbp.tile([P, ET, P], F32R, tag="ktb")
                    nc.sync.dma_start(ktb[:], ktd3[:, :, kb * P:(kb + 1) * P])
                    ps = psp.tile([P, QC], F32, tag="ps_s")
                    for et in range(ET):
                        nc.tensor.matmul(
                            ps[:], ktb[:, et, :], qt_sb[:, et, q0:q0 + QC],
                            start=(et == 0), stop=(et == ET - 1))
                    nc.scalar.activation(es[:, kb, :], ps[:], EXP, scale=SCALE)
                # denominator over the k (partition) axis via ones-matmul
                pd = pdp.tile([1, QC], F32, tag="ps_d")
                for kb in range(KB):
                    nc.tensor.matmul(pd[:], onesk[:], es[:, kb, :],
                                     start=(kb == 0), stop=(kb == KB - 1))
                recd = recp.tile([1, QC], F32R, tag="recd")
                with nc.allow_low_precision(reason="f32r feed for broadcast matmul"):
                    nc.vector.reciprocal(recd[:], pd[:])
                # broadcast 1/denom across 128 partitions with a K=1 matmul
                pb = pdp.tile([P, QC], F32, tag="ps_b")
                nc.tensor.matmul(pb[:], ones1[:], recd[:], start=True, stop=True)
                recb = recp.tile([P, QC], F32, tag="recb")
                nc.vector.tensor_copy(recb[:], pb[:])
                # U^T accumulation + normalized copy-out
                for et in range(ET):
                    pu = pup.tile([P, QC], F32, tag="ps_u")
                    for kb in range(KB):
                        nc.tensor.matmul(
                            pu[:], v_sb[:, kb, et * P:(et + 1) * P], es[:, kb, :],
                            start=(kb == 0), stop=(kb == KB - 1))
                    nc.vector.tensor_tensor(
                        ot_sb[:, et, q0:q0 + QC], pu[:], recb[:], MULT)

        # ---------------- Phase F: fc + bias + residual ----------------
        with tc.tile_pool(name="wf", bufs=1) as wfp, \
             tc.tile_pool(name="xrt", bufs=2) as xrp, \
             tc.tile_pool(name="ysb", bufs=3) as ysp, \
             tc.tile_pool(name="py", bufs=3, space="PSUM") as pyp:
            wf_sb = wfp.tile([P, DT, DIM], F32R)
            nc.sync.dma_start(wf_sb[:], wf_d[:].rearrange("(dt p) e -> p dt e", p=P))
            for q_t in range(HALF // P):
                xrt = xrp.tile([P, DIM], F32, tag="xrt")
                nc.sync.dma_start(xrt[:], xr_d[q_t * P:(q_t + 1) * P, :])
                for ec in range(2):
                    ps = pyp.tile([P, 512], F32, tag="py")
                    for dt in range(DT):
                        nc.tensor.matmul(
                            ps[:], ot_sb[:, dt, q_t * P:(q_t + 1) * P],
                            wf_sb[:, dt, ec * 512:(ec + 1) * 512],
                            start=(dt == 0), stop=(dt == DT - 1))
                    ysb = ysp.tile([P, 512], F32, tag="ysb")
                    nc.vector.tensor_tensor(
                        ysb[:], ps[:], bfb[:, ec * 512:(ec + 1) * 512], ADD)
                    nc.vector.tensor_tensor(
                        ysb[:], ysb[:], xrt[:, ec * 512:(ec + 1) * 512], ADD)
                    nc.sync.dma_start(
                        y_d[q_t * P:(q_t + 1) * P, ec * 512:(ec + 1) * 512], ysb[:])

        opool.release()
        qpool.release()
        vpool.release()
        cpool.release()

    nc.finalize()
    return nc


def _get_nc():
    if "nc" not in _CACHE:
        _CACHE["nc"] = _build()
    return _CACHE["nc"]


def kernel(x, Wq, bq, Wk, bk, Wv, bv, Wfc, bfc):
    from concourse.bass_utils import run_bass_kernel_spmd

    x = np.asarray(x, dtype=np.float32)
    nc = _get_nc()

    wqT = np.ascontiguousarray(np.asarray(Wq, np.float32).T)
    wkT = np.ascontiguousarray(np.asarray(Wk, np.float32).T)
    wvT = np.ascontiguousarray(np.asarray(Wv, np.float32).T)
    wfT = np.ascontiguousarray(np.asarray(Wfc, np.float32).T)
    bq = np.asarray(bq, np.float32); bk = np.asarray(bk, np.float32)
    bv = np.asarray(bv, np.float32); bf = np.asarray(bfc, np.float32)

    in_maps = []
    for core in range(NCORES):
        b, h = core // 2, core % 2
        xtb = np.ascontiguousarray(x[b].T)  # [DIM, S]
        # roll so this core's q-half sits at columns [0, HALF)
        xt = np.ascontiguousarray(np.roll(xtb, -h * HALF, axis=1)) if h else xtb
        in_maps.append({
            "xt": xt,
            "xr": np.ascontiguousarray(x[b, h * HALF:(h + 1) * HALF, :]),
            "wq": wqT, "wk": wkT, "wv": wvT, "wf": wfT,
            "bq": bq, "bk": bk, "bv": bv, "bf": bf,
        })

    res = run_bass_kernel_spmd(nc, in_maps, core_ids=list(range(NCORES)))
    out = np.empty((B, S, DIM), dtype=np.float32)
    for core in range(NCORES):
        b, h = core // 2, core % 2
        out[b, h * HALF:(h + 1) * HALF, :] = res.results[core]["y"]
    return out


# revision 25
# speedup vs baseline: 202.7589x; 202.7589x over previous
"""AttentionBlock kernel for 8 Trainium2 NeuronCores.

Problem (hardcoded): x [4, 2048, 1024] f32; Wq/Wk/Wv/Wfc [1024, 1024]; biases [1024].
    q = x@Wq.T+bq; k = x@Wk.T+bk; v = x@Wv.T+bv
    out = softmax(q k^T / sqrt(1024)) v;  y = out@Wfc.T+bfc + x

Sharding: core i = (b = i//2, h = i%2). Each core computes the full V / scores for
its batch element (duplicated across the 2 cores sharing a batch) and the
attention + fc for its half of the sequence. No collectives (measured ~40us fixed
+ ~7.6us/MB per 2-core AllGather here -- a K/V exchange costs more than it saves).

Key algebraic trick: q k^T = x (Wq^T Wk) x^T, so the host pre-contracts
M = Wq^T @ Wk and the kernel never materializes Q or K:
    G^T = M-blocks^T @ xT           (27us instead of Q-proj 27 + K-proj 55)
    S^T = xT-blocks^T @ G^T         (55us, lhsT streamed straight from x!)
The bias cross-terms are exact: the per-q term and constant cancel in softmax;
the per-k term r2[k] = x_k . (Wk^T bq) is a cheap rank-1 matmul folded into the
exp's per-partition bias.

Per-core plan (all matmuls float32r = full PE rate, ~2e-4 rel err):
  host feeds xT = x[b].T (d-major, rolled so this core's q-half is columns 0:1024)
  plus M, Wv^T, Wfc^T, so every GEMM has its contraction dim on partitions with
  no on-device transposes.
  - G^T [d, q] resident; V [s, e] resident (one xT sweep); r2 column per k-block
  - attention per q-chunk of 512: S^T blocks with xT streamed as lhsT, softmax
    over the partition (k) axis: exp(scale*S + r2) without max-subtract
    (|S|*scale <~ 6 here), denominator via ones-matmul, reciprocal broadcast
    across partitions with a rank-1 PE matmul, U^T = V-block.T @ expS^T
    accumulated in PSUM and normalized on copy-out -> O^T spilled to DRAM
  - fc: y = (O^T-block).T @ Wfc^T + bfc + x
"""

import numpy as np

B, S, DIM = 4, 2048, 1024
P = 128
NCORES = 8
HALF = S // 2          # 1024 q rows per core
DT = DIM // P          # 8 d tiles
ET = DIM // P          # 8 e tiles
SCH = S // 512         # 4 s-chunks for the V sweep
QC = 512               # attention q-chunk
NQ = HALF // QC        # 2 q chunks
KB = S // P            # 16 k blocks
SCALE = 1.0 / float(np.sqrt(DIM))

_CACHE = {}
TIMING_REPEAT = 21


def _build(repeat=1):
    import concourse.mybir as mybir
    import concourse.tile as tile
    from concourse import bacc

    F32 = mybir.dt.float32
    F32R = mybir.dt.float32r
    EXP = mybir.ActivationFunctionType.Exp
    IDENT = mybir.ActivationFunctionType.Identity
    ADD = mybir.AluOpType.add
    MULT = mybir.AluOpType.mult

    nc = bacc.Bacc()

    xt_d = nc.dram_tensor("xt", [DIM, S], F32R, kind="ExternalInput")
    xr_d = nc.dram_tensor("xr", [HALF, DIM], F32, kind="ExternalInput")
    m_d = nc.dram_tensor("m", [DIM, DIM], F32R, kind="ExternalInput")   # Wq^T Wk
    wv_d = nc.dram_tensor("wv", [DIM, DIM], F32R, kind="ExternalInput")
    wf_d = nc.dram_tensor("wf", [DIM, DIM], F32R, kind="ExternalInput")
    c2_d = nc.dram_tensor("c2", [DIM, 2], F32R, kind="ExternalInput")   # Wk^T bq, x2
    bv_d = nc.dram_tensor("bv", [DIM], F32, kind="ExternalInput")
    bf_d = nc.dram_tensor("bf", [DIM], F32, kind="ExternalInput")
    y_d = nc.dram_tensor("y", [HALF, DIM], F32, kind="ExternalOutput")
    ot_d = nc.dram_tensor("otspill", [ET, P, HALF], F32R)  # O^T spill [et, e_p, q]

    xt3 = xt_d[:].rearrange("(dt p) s -> p dt s", p=P)      # [128, 8, 2048]
    otd3 = ot_d[:].rearrange("et p q -> p et q")            # [128, 8, 1024]
    m3 = m_d[:].rearrange("(dt p) e -> p dt e", p=P)
    wv3 = wv_d[:].rearrange("(dt p) e -> p dt e", p=P)
    wf3 = wf_d[:].rearrange("(dt p) e -> p dt e", p=P)
    c23 = c2_d[:].rearrange("(t p) w -> p t w", p=P)        # [128, 8, 2]

    with tile.TileContext(nc, pool_alloc_mode="stack") as tc:
        cpool = tc.alloc_tile_pool(name="const", bufs=1)
        onesk = cpool.tile([P, 1], F32R)   # denominator lhsT
        ones1 = cpool.tile([1, P], F32R)   # partition-broadcast lhsT
        ones_f32 = cpool.tile([P, P], F32)
        # Wk^T bq as columns per d-tile, duplicated x2 (fp32r matmuls need an
        # even moving free count)
        c2c = cpool.tile([P, DT, 2], F32R)
        nc.scalar.dma_start(c2c[:], c23)
        nc.vector.memset(ones_f32[:], 1.0)
        nc.vector.tensor_copy(onesk[:], ones_f32[:, 0:1])
        nc.vector.tensor_copy(ones1[:], ones_f32[0:1, :])
        # warm the ACT LUTs (first use otherwise pays a ~1.4us cold table load)
        warm = cpool.tile([1, 2], F32)
        nc.scalar.activation(warm[0:1, 0:1], ones_f32[0:1, 0:1], IDENT)
        nc.scalar.activation(warm[0:1, 1:2], ones_f32[0:1, 0:1], EXP)

        for _rep in range(repeat):
            # -------- Phase G: G^T = (Wq^T Wk)-blocks^T @ xT-half (resident) ----
            gpool = tc.alloc_tile_pool(name="gt", bufs=1)
            gt_sb = gpool.tile([P, DT, HALF], F32R, tag="gt")  # [d_p, d_tile, q]
            with tc.tile_pool(name="mq", bufs=1) as mqp, \
                 tc.tile_pool(name="xtq", bufs=2) as xtqp, \
                 tc.tile_pool(name="pq", bufs=3, space="PSUM") as pqp:
                m_sb = mqp.tile([P, DT, DIM], F32R)
                xtq0 = xtqp.tile([P, DT, 512], F32R, tag="xtq")
                # interleave the first loads across all three DMA queues so the
                # first group isn't gated by one queue's serial transfer rate
                engs = (nc.sync, nc.scalar, nc.gpsimd)
                for dt in range(DT):
                    engs[(2 * dt) % 3].dma_start(m_sb[:, dt, :], m3[:, dt, :])
                    engs[(2 * dt + 1) % 3].dma_start(xtq0[:, dt, :], xt3[:, dt, 0:512])
                for qch in range(HALF // 512):
                    if qch == 0:
                        xtq = xtq0
                    else:
                        xtq = xtqp.tile([P, DT, 512], F32R, tag="xtq")
                        nc.sync.dma_start(xtq[:], xt3[:, :, qch * 512:(qch + 1) * 512])
                    for dtile in range(DT):
                        ps = pqp.tile([P, 512], F32, tag="pq")
                        for dt in range(DT):
                            nc.tensor.matmul(
                                ps[:], m_sb[:, dt, dtile * P:(dtile + 1) * P],
                                xtq[:, dt, :],
                                start=(dt == 0), stop=(dt == DT - 1))
                        nc.scalar.activation(
                            gt_sb[:, dtile, qch * 512:(qch + 1) * 512], ps[:], IDENT)

            # -------- Phase V: V -> SBUF resident + r2 columns (one xT sweep) ---
            vpool = tc.alloc_tile_pool(name="v", bufs=1)
            v_sb = vpool.tile([P, KB, DIM], F32R, tag="v")  # [s_p, s_tile, e]
            bvb = vpool.tile([P, DIM], F32, tag="bvb")
            r2c = vpool.tile([P, KB], F32, tag="r2c")  # scale*(x_k . Wk^T bq) per kb
            nc.scalar.dma_start(bvb[:], bv_d[:][None, :].to_broadcast((P, DIM)))
            with tc.tile_pool(name="wvp", bufs=1) as wvp, \
                 tc.tile_pool(name="xtk", bufs=2) as xtkp, \
                 tc.tile_pool(name="pkv", bufs=3, space="PSUM") as pkvp, \
                 tc.tile_pool(name="pr2", bufs=2, space="PSUM") as pr2p:
                wv_sb = wvp.tile([P, DT, DIM], F32R, tag="wv")
                xtk0 = xtkp.tile([P, DT, 512], F32R, tag="xtk")
                for dt in range(DT):
                    nc.sync.dma_start(wv_sb[:, dt, :], wv3[:, dt, :])
                    nc.gpsimd.dma_start(xtk0[:, dt, :], xt3[:, dt, 0:512])
                for sch in range(SCH):
                    s0 = sch * 512
                    if sch == 0:
                        xtk = xtk0
                    else:
                        xtk = xtkp.tile([P, DT, 512], F32R, tag="xtk")
                        nc.sync.dma_start(xtk[:], xt3[:, :, s0:s0 + 512])
                    for st4 in range(4):
                        st = sch * 4 + st4
                        for eh in range(2):
                            ps = pkvp.tile([P, 512], F32, tag="pv")
                            for dt in range(DT):
                                nc.tensor.matmul(
                                    ps[:], xtk[:, dt, st4 * P:(st4 + 1) * P],
                                    wv_sb[:, dt, eh * 512:(eh + 1) * 512],
                                    start=(dt == 0), stop=(dt == DT - 1))
                            nc.vector.tensor_tensor(
                                v_sb[:, st, eh * 512:(eh + 1) * 512], ps[:],
                                bvb[:, eh * 512:(eh + 1) * 512], ADD)
                        # r2 column for this k-block (exact bias cross-term)
                        pr = pr2p.tile([P, 2], F32, tag="pr2")
                        for dt in range(DT):
                            nc.tensor.matmul(
                                pr[:], xtk[:, dt, st4 * P:(st4 + 1) * P],
                                c2c[:, dt, :],
                                start=(dt == 0), stop=(dt == DT - 1))
                        nc.scalar.activation(r2c[:, st:st + 1], pr[:, 0:1], IDENT,
                                             scale=SCALE)

            # ---------------- Phase A: attention (per q-chunk of 512) -----------
            espool = tc.alloc_tile_pool(name="es", bufs=1)
            wfpool = tc.alloc_tile_pool(name="wf", bufs=2)
            xtbpool = tc.alloc_tile_pool(name="xtb", bufs=3)
            wf_sb = wfpool.tile([P, DT, DIM], F32R, tag="wf", bufs=1)
            bfb = wfpool.tile([P, DIM], F32, tag="bfb", bufs=1)
            nc.gpsimd.dma_start(bfb[:], bf_d[:][None, :].to_broadcast((P, DIM)))
            nc.gpsimd.dma_start(wf_sb[:], wf3[:])  # prefetch during attention
            with tc.tile_pool(name="rec", bufs=2) as recp, \
                 tc.tile_pool(name="otc", bufs=3) as otcp, \
                 tc.tile_pool(name="ps_s", bufs=2, space="PSUM") as psp, \
                 tc.tile_pool(name="ps_u", bufs=2, space="PSUM") as pup, \
                 tc.tile_pool(name="ps_d", bufs=2, space="PSUM") as pdp:
                for qc in range(NQ):
                    q0 = qc * QC
                    es = espool.tile([P, KB, QC], F32R, tag="es")  # exp [k_p, kb, q]
                    for kb in range(KB):
                        xtb = xtbpool.tile([P, DT, P], F32R, tag="xtb")
                        nc.sync.dma_start(xtb[:], xt3[:, :, kb * P:(kb + 1) * P])
                        ps = psp.tile([P, QC], F32, tag="ps_s")
                        for dt in range(DT):
                            nc.tensor.matmul(
                                ps[:], xtb[:, dt, :], gt_sb[:, dt, q0:q0 + QC],
                                start=(dt == 0), stop=(dt == DT - 1))
                        nc.scalar.activation(es[:, kb, :], ps[:], EXP,
                                             bias=r2c[:, kb:kb + 1], scale=SCALE)
                    # denominator over the k (partition) axis via ones-matmul
                    pd = pdp.tile([1, QC], F32, tag="ps_d")
                    for kb in range(KB):
                        nc.tensor.matmul(pd[:], onesk[:], es[:, kb, :],
                                         start=(kb == 0), stop=(kb == KB - 1))
                    recd = recp.tile([1, QC], F32R, tag="recd")
                    with nc.allow_low_precision(reason="f32r feed for broadcast matmul"):
                        nc.vector.reciprocal(recd[:], pd[:])
                    # broadcast 1/denom across 128 partitions with a K=1 matmul
                    pb = pdp.tile([P, QC], F32, tag="ps_b")
                    nc.tensor.matmul(pb[:], ones1[:], recd[:], start=True, stop=True)
                    recb = recp.tile([P, QC], F32, tag="recb")
                    nc.vector.tensor_copy(recb[:], pb[:])
                    # U^T accumulation + normalized copy-out -> DRAM spill
                    for et in range(ET):
                        pu = pup.tile([P, QC], F32, tag="ps_u")
                        for kb in range(KB):
                            nc.tensor.matmul(
                                pu[:], v_sb[:, kb, et * P:(et + 1) * P], es[:, kb, :],
                                start=(kb == 0), stop=(kb == KB - 1))
                        otc = otcp.tile([P, QC], F32R, tag="otc")
                        nc.vector.tensor_tensor(otc[:], pu[:], recb[:], MULT)
                        nc.gpsimd.dma_start(ot_d[et, :, q0:q0 + QC], otc[:])

            xtbpool.release()

            # ---------------- Phase F: fc + bias + residual ----------------
            with tc.tile_pool(name="xrt", bufs=2) as xrp, \
                 tc.tile_pool(name="ysb", bufs=4) as ysp, \
                 tc.tile_pool(name="py", bufs=3, space="PSUM") as pyp:
                for q_t in range(HALF // P):
                    xrt = xrp.tile([P, DIM], F32, tag="xrt")
                    nc.scalar.dma_start(xrt[:], xr_d[q_t * P:(q_t + 1) * P, :])
                    otb = wfpool.tile([P, DT, P], F32R, tag="otb")
                    nc.sync.dma_start(otb[:], otd3[:, :, q_t * P:(q_t + 1) * P])
                    for ec in range(2):
                        ps = pyp.tile([P, 512], F32, tag="py")
                        for dt in range(DT):
                            nc.tensor.matmul(
                                ps[:], otb[:, dt, :],
                                wf_sb[:, dt, ec * 512:(ec + 1) * 512],
                                start=(dt == 0), stop=(dt == DT - 1))
                        ysb = ysp.tile([P, 512], F32, tag="ysb")
                        nc.vector.tensor_tensor(
                            ysb[:], ps[:], bfb[:, ec * 512:(ec + 1) * 512], ADD)
                        nc.vector.tensor_tensor(
                            ysb[:], ysb[:], xrt[:, ec * 512:(ec + 1) * 512], ADD)
                        nc.gpsimd.dma_start(
                            y_d[q_t * P:(q_t + 1) * P, ec * 512:(ec + 1) * 512], ysb[:])

            wfpool.release()
            espool.release()
            vpool.release()
            gpool.release()
        cpool.release()

    nc.finalize()
    return nc


def _get_nc():
    if "nc" not in _CACHE:
        _CACHE["nc"] = _build()
    return _CACHE["nc"]


def _make_in_maps(x, Wq, bq, Wk, bk, Wv, bv, Wfc, bfc):
    x = np.asarray(x, dtype=np.float32)
    Wq = np.asarray(Wq, np.float32); Wk = np.asarray(Wk, np.float32)
    m = np.ascontiguousarray(Wq.T @ Wk)            # q k^T = x m x^T
    c2v = Wk.T @ np.asarray(bq, np.float32)
    c2 = np.ascontiguousarray(np.repeat(c2v[:, None], 2, axis=1))
    wvT = np.ascontiguousarray(np.asarray(Wv, np.float32).T)
    wfT = np.ascontiguousarray(np.asarray(Wfc, np.float32).T)
    bv = np.asarray(bv, np.float32); bf = np.asarray(bfc, np.float32)

    in_maps = []
    for core in range(NCORES):
        b, h = core // 2, core % 2
        xtb = np.ascontiguousarray(x[b].T)  # [DIM, S]
        # roll so this core's q-half sits at columns [0, HALF); the k ordering
        # permutes consistently in scores and V, and softmax+sum over k is
        # permutation-invariant, so one SPMD program serves both halves.
        xt = np.ascontiguousarray(np.roll(xtb, -h * HALF, axis=1)) if h else xtb
        in_maps.append({
            "xt": xt,
            "xr": np.ascontiguousarray(x[b, h * HALF:(h + 1) * HALF, :]),
            "m": m, "wv": wvT, "wf": wfT,
            "c2": c2, "bv": bv, "bf": bf,
        })
    return in_maps


def kernel(x, Wq, bq, Wk, bk, Wv, bv, Wfc, bfc):
    from concourse.bass_utils import run_bass_kernel_spmd

    nc = _get_nc()
    in_maps = _make_in_maps(x, Wq, bq, Wk, bk, Wv, bv, Wfc, bfc)
    res = run_bass_kernel_spmd(nc, in_maps, core_ids=list(range(NCORES)))
    out = np.empty((B, S, DIM), dtype=np.float32)
    for core in range(NCORES):
        b, h = core // 2, core % 2
        out[b, h * HALF:(h + 1) * HALF, :] = res.results[core]["y"]
    return out


# revision 28
# speedup vs baseline: 362.9662x; 1.7901x over previous
"""AttentionBlock kernel for 8 Trainium2 NeuronCores.

Problem (hardcoded): x [4, 2048, 1024] f32; Wq/Wk/Wv/Wfc [1024, 1024]; biases [1024].
    q = x@Wq.T+bq; k = x@Wk.T+bk; v = x@Wv.T+bv
    out = softmax(q k^T / sqrt(1024)) v;  y = out@Wfc.T+bfc + x

Sharding: core i = (b = i//2, h = i%2). Each core computes the full V / scores for
its batch element (duplicated across the 2 cores sharing a batch) and the
attention + fc for its half of the sequence. No collectives (measured ~40us fixed
+ ~7.6us/MB per 2-core AllGather here -- a K/V exchange costs more than it saves).

Key algebraic trick: q k^T = x (Wq^T Wk) x^T, so the host pre-contracts
M = Wq^T @ Wk and the kernel never materializes Q or K:
    G^T = M-blocks^T @ xT           (27us instead of Q-proj 27 + K-proj 55)
    S^T = xT-blocks^T @ G^T         (55us, lhsT streamed straight from x!)
The bias cross-terms are exact: the per-q term and constant cancel in softmax;
the per-k term r2[k] = x_k . (Wk^T bq) is a cheap rank-1 matmul folded into the
exp's per-partition bias.

Per-core plan (all matmuls float32r = full PE rate, ~2e-4 rel err):
  host feeds xT = x[b].T (d-major, rolled so this core's q-half is columns 0:1024)
  plus M, Wv^T, Wfc^T, so every GEMM has its contraction dim on partitions with
  no on-device transposes.
  - G^T [d, q] resident; V [s, e] resident (one xT sweep); r2 column per k-block
  - attention per q-chunk of 512: S^T blocks with xT streamed as lhsT, softmax
    over the partition (k) axis: exp(scale*S + r2) without max-subtract
    (|S|*scale <~ 6 here), denominator via ones-matmul, reciprocal broadcast
    across partitions with a rank-1 PE matmul, U^T = V-block.T @ expS^T
    accumulated in PSUM and normalized on copy-out -> O^T spilled to DRAM
  - fc: y = (O^T-block).T @ Wfc^T + bfc + x
"""

import numpy as np

B, S, DIM = 4, 2048, 1024
P = 128
NCORES = 8
HALF = S // 2          # 1024 q rows per core
DT = DIM // P          # 8 d tiles
ET = DIM // P          # 8 e tiles
SCH = S // 512         # 4 s-chunks for the V sweep
QC = 512               # attention q-chunk
NQ = HALF // QC        # 2 q chunks
KB = S // P            # 16 k blocks
SCALE = 1.0 / float(np.sqrt(DIM))

_CACHE = {}
TIMING_REPEAT = 21


def _build(repeat=1):
    import concourse.mybir as mybir
    import concourse.tile as tile
    from concourse import bacc

    F32 = mybir.dt.float32
    F32R = mybir.dt.float32r
    EXP = mybir.ActivationFunctionType.Exp
    IDENT = mybir.ActivationFunctionType.Identity
    ADD = mybir.AluOpType.add
    MULT = mybir.AluOpType.mult

    nc = bacc.Bacc()

    xt_d = nc.dram_tensor("xt", [DIM, S], F32R, kind="ExternalInput")
    xr_d = nc.dram_tensor("xr", [HALF, DIM], F32, kind="ExternalInput")
    m_d = nc.dram_tensor("m", [DIM, DIM], F32R, kind="ExternalInput")   # Wq^T Wk
    n_d = nc.dram_tensor("n", [DIM, DIM], F32R, kind="ExternalInput")   # Wv^T Wfc^T
    c2_d = nc.dram_tensor("c2", [DIM, 2], F32R, kind="ExternalInput")   # Wk^T bq, x2
    bvf_d = nc.dram_tensor("bvf", [DIM], F32, kind="ExternalInput")     # Wfc @ bv
    bf_d = nc.dram_tensor("bf", [DIM], F32, kind="ExternalInput")
    y_d = nc.dram_tensor("y", [HALF, DIM], F32, kind="ExternalOutput")

    xt3 = xt_d[:].rearrange("(dt p) s -> p dt s", p=P)      # [128, 8, 2048]
    m3 = m_d[:].rearrange("(dt p) e -> p dt e", p=P)
    n3 = n_d[:].rearrange("(dt p) e -> p dt e", p=P)
    c23 = c2_d[:].rearrange("(t p) w -> p t w", p=P)        # [128, 8, 2]

    with tile.TileContext(nc, pool_alloc_mode="stack") as tc:
        cpool = tc.alloc_tile_pool(name="const", bufs=1)
        ones2 = cpool.tile([P, 2], F32R)   # denominator rhs (even-N fp32r rule)
        ones_f32 = cpool.tile([P, P], F32)
        # Wk^T bq as columns per d-tile, duplicated x2 (fp32r matmuls need an
        # even moving free count)
        c2c = cpool.tile([P, DT, 2], F32R)
        nc.scalar.dma_start(c2c[:], c23)
        nc.vector.memset(ones_f32[:], 1.0)
        nc.vector.tensor_copy(ones2[:], ones_f32[:, 0:2])
        # warm the ACT LUTs (first use otherwise pays a ~1.4us cold table load)
        warm = cpool.tile([1, 2], F32)
        nc.scalar.activation(warm[0:1, 0:1], ones_f32[0:1, 0:1], IDENT)
        nc.scalar.activation(warm[0:1, 1:2], ones_f32[0:1, 0:1], EXP)

        for _rep in range(repeat):
            # -------- Phase G: G^T = (Wq^T Wk)-blocks^T @ xT-half (resident) ----
            gpool = tc.alloc_tile_pool(name="gt", bufs=1)
            gt_sb = gpool.tile([P, DT, HALF], F32R, tag="gt")  # [d_p, d_tile, q]
            with tc.tile_pool(name="mq", bufs=1) as mqp, \
                 tc.tile_pool(name="xtq", bufs=2) as xtqp, \
                 tc.tile_pool(name="pq", bufs=3, space="PSUM") as pqp:
                m_sb = mqp.tile([P, DT, DIM], F32R)
                xtq0 = xtqp.tile([P, DT, 512], F32R, tag="xtq")
                # interleave the first loads across all three DMA queues so the
                # first group isn't gated by one queue's serial transfer rate
                engs = (nc.sync, nc.scalar, nc.gpsimd)
                for dt in range(DT):
                    engs[(2 * dt) % 3].dma_start(m_sb[:, dt, :], m3[:, dt, :])
                    engs[(2 * dt + 1) % 3].dma_start(xtq0[:, dt, :], xt3[:, dt, 0:512])
                for qch in range(HALF // 512):
                    if qch == 0:
                        xtq = xtq0
                    else:
                        xtq = xtqp.tile([P, DT, 512], F32R, tag="xtq")
                        nc.sync.dma_start(xtq[:], xt3[:, :, qch * 512:(qch + 1) * 512])
                    for dtile in range(DT):
                        ps = pqp.tile([P, 512], F32, tag="pq")
                        for dt in range(DT):
                            nc.tensor.matmul(
                                ps[:], m_sb[:, dt, dtile * P:(dtile + 1) * P],
                                xtq[:, dt, :],
                                start=(dt == 0), stop=(dt == DT - 1))
                        nc.scalar.activation(
                            gt_sb[:, dtile, qch * 512:(qch + 1) * 512], ps[:], IDENT)

            # ---- Phase VF: VF = x @ (Wv^T Wfc^T) + Wfc@bv -> SBUF resident ----
            # (P/denom) @ VF is then the fc output directly: the whole fc phase
            # and the O^T spill disappear. r2 columns computed in the same sweep.
            vpool = tc.alloc_tile_pool(name="vf", bufs=1)
            vf_sb = vpool.tile([P, KB, DIM], F32R, tag="vf")  # [s_p, s_tile, e2]
            bvfb = vpool.tile([P, DIM], F32, tag="bvfb")
            r2c = vpool.tile([P, KB], F32, tag="r2c")  # scale*(x_k . Wk^T bq) per kb
            nc.scalar.dma_start(bvfb[:], bvf_d[:][None, :].to_broadcast((P, DIM)))
            with tc.tile_pool(name="wvp", bufs=1) as wvp, \
                 tc.tile_pool(name="xtk", bufs=2) as xtkp, \
                 tc.tile_pool(name="pkv", bufs=3, space="PSUM") as pkvp, \
                 tc.tile_pool(name="pr2", bufs=2, space="PSUM") as pr2p:
                n_sb = wvp.tile([P, DT, DIM], F32R, tag="n")
                xtk0 = xtkp.tile([P, DT, 512], F32R, tag="xtk")
                for dt in range(DT):
                    nc.sync.dma_start(n_sb[:, dt, :], n3[:, dt, :])
                    nc.gpsimd.dma_start(xtk0[:, dt, :], xt3[:, dt, 0:512])
                for sch in range(SCH):
                    s0 = sch * 512
                    if sch == 0:
                        xtk = xtk0
                    else:
                        xtk = xtkp.tile([P, DT, 512], F32R, tag="xtk")
                        nc.sync.dma_start(xtk[:], xt3[:, :, s0:s0 + 512])
                    for st4 in range(4):
                        st = sch * 4 + st4
                        for eh in range(2):
                            ps = pkvp.tile([P, 512], F32, tag="pv")
                            for dt in range(DT):
                                nc.tensor.matmul(
                                    ps[:], xtk[:, dt, st4 * P:(st4 + 1) * P],
                                    n_sb[:, dt, eh * 512:(eh + 1) * 512],
                                    start=(dt == 0), stop=(dt == DT - 1))
                            nc.vector.tensor_tensor(
                                vf_sb[:, st, eh * 512:(eh + 1) * 512], ps[:],
                                bvfb[:, eh * 512:(eh + 1) * 512], ADD)
                        # r2 column for this k-block (exact bias cross-term)
                        pr = pr2p.tile([P, 2], F32, tag="pr2")
                        for dt in range(DT):
                            nc.tensor.matmul(
                                pr[:], xtk[:, dt, st4 * P:(st4 + 1) * P],
                                c2c[:, dt, :],
                                start=(dt == 0), stop=(dt == DT - 1))
                        nc.scalar.activation(r2c[:, st:st + 1], pr[:, 0:1], IDENT,
                                             scale=SCALE)

            # ------- Phase A: attention -> y directly (per q-chunk of 512) ------
            # S^T blocks -> exp -> per-q denominator columns; then
            # psum_y[q, e2] = sum_kb es-block^T @ VF-block gives the fc output in
            # natural layout (es is the stationary operand), normalized by a
            # per-partition 1/denom scale on the ACT copy-out, + bfc + x.
            espool = tc.alloc_tile_pool(name="es", bufs=1)
            xtbpool = tc.alloc_tile_pool(name="xtb", bufs=3)
            bfbp = tc.alloc_tile_pool(name="bfbp", bufs=1)
            bfb = bfbp.tile([P, DIM], F32)
            nc.gpsimd.dma_start(bfb[:], bf_d[:][None, :].to_broadcast((P, DIM)))
            with tc.tile_pool(name="rec", bufs=2) as recp, \
                 tc.tile_pool(name="xrt", bufs=3) as xrp, \
                 tc.tile_pool(name="ysb", bufs=4) as ysp, \
                 tc.tile_pool(name="ps_s", bufs=2, space="PSUM") as psp, \
                 tc.tile_pool(name="ps_y", bufs=3, space="PSUM") as pyp, \
                 tc.tile_pool(name="ps_d", bufs=2, space="PSUM") as pdp:
                for qc in range(NQ):
                    q0 = qc * QC
                    es = espool.tile([P, KB, QC], F32R, tag="es")  # exp [k_p, kb, q]
                    for kb in range(KB):
                        xtb = xtbpool.tile([P, DT, P], F32R, tag="xtb")
                        nc.sync.dma_start(xtb[:], xt3[:, :, kb * P:(kb + 1) * P])
                        ps = psp.tile([P, QC], F32, tag="ps_s")
                        for dt in range(DT):
                            nc.tensor.matmul(
                                ps[:], xtb[:, dt, :], gt_sb[:, dt, q0:q0 + QC],
                                start=(dt == 0), stop=(dt == DT - 1))
                        nc.scalar.activation(es[:, kb, :], ps[:], EXP,
                                             bias=r2c[:, kb:kb + 1], scale=SCALE)
                    # per-q denominator columns + reciprocal, one per q-block
                    recq = recp.tile([P, QC // P], F32, tag="recq")
                    for qb in range(QC // P):
                        pd = pdp.tile([P, 2], F32, tag="ps_d")
                        for kb in range(KB):
                            nc.tensor.matmul(
                                pd[:], es[:, kb, qb * P:(qb + 1) * P], ones2[:],
                                start=(kb == 0), stop=(kb == KB - 1))
                        with nc.allow_low_precision(reason="per-partition scale vec"):
                            nc.vector.reciprocal(recq[:, qb:qb + 1], pd[:, 0:1])
                    # y = (es/denom)^T @ VF + bfc + x, written straight out
                    for qb in range(QC // P):
                        q_t = qc * (QC // P) + qb
                        xrt = xrp.tile([P, DIM], F32, tag="xrt")
                        nc.scalar.dma_start(xrt[:], xr_d[q_t * P:(q_t + 1) * P, :])
                        for ec in range(2):
                            py = pyp.tile([P, 512], F32, tag="ps_y")
                            for kb in range(KB):
                                nc.tensor.matmul(
                                    py[:], es[:, kb, qb * P:(qb + 1) * P],
                                    vf_sb[:, kb, ec * 512:(ec + 1) * 512],
                                    start=(kb == 0), stop=(kb == KB - 1))
                            ysb = ysp.tile([P, 512], F32, tag="ysb")
                            nc.scalar.activation(ysb[:], py[:], IDENT,
                                                 scale=recq[:, qb:qb + 1])
                            nc.vector.tensor_tensor(
                                ysb[:], ysb[:], bfb[:, ec * 512:(ec + 1) * 512], ADD)
                            nc.vector.tensor_tensor(
                                ysb[:], ysb[:], xrt[:, ec * 512:(ec + 1) * 512], ADD)
                            nc.gpsimd.dma_start(
                                y_d[q_t * P:(q_t + 1) * P, ec * 512:(ec + 1) * 512],
                                ysb[:])

            bfbp.release()
            xtbpool.release()
            espool.release()
            vpool.release()
            gpool.release()
        cpool.release()

    nc.finalize()
    return nc


def _get_nc():
    if "nc" not in _CACHE:
        _CACHE["nc"] = _build()
    return _CACHE["nc"]


def _make_in_maps(x, Wq, bq, Wk, bk, Wv, bv, Wfc, bfc):
    x = np.asarray(x, dtype=np.float32)
    Wq = np.asarray(Wq, np.float32); Wk = np.asarray(Wk, np.float32)
    Wv = np.asarray(Wv, np.float32); Wfc = np.asarray(Wfc, np.float32)
    m = np.ascontiguousarray(Wq.T @ Wk)            # q k^T = x m x^T
    n = np.ascontiguousarray(Wv.T @ Wfc.T)         # (P/denom) @ (x n) = fc out
    c2v = Wk.T @ np.asarray(bq, np.float32)
    c2 = np.ascontiguousarray(np.repeat(c2v[:, None], 2, axis=1))
    bvf = np.ascontiguousarray(Wfc @ np.asarray(bv, np.float32))
    bf = np.asarray(bfc, np.float32)

    in_maps = []
    for core in range(NCORES):
        b, h = core // 2, core % 2
        xtb = np.ascontiguousarray(x[b].T)  # [DIM, S]
        # roll so this core's q-half sits at columns [0, HALF); the k ordering
        # permutes consistently in scores and V, and softmax+sum over k is
        # permutation-invariant, so one SPMD program serves both halves.
        xt = np.ascontiguousarray(np.roll(xtb, -h * HALF, axis=1)) if h else xtb
        in_maps.append({
            "xt": xt,
            "xr": np.ascontiguousarray(x[b, h * HALF:(h + 1) * HALF, :]),
            "m": m, "n": n, "c2": c2, "bvf": bvf, "bf": bf,
        })
    return in_maps


def kernel(x, Wq, bq, Wk, bk, Wv, bv, Wfc, bfc):
    from concourse.bass_utils import run_bass_kernel_spmd

    nc = _get_nc()
    in_maps = _make_in_maps(x, Wq, bq, Wk, bk, Wv, bv, Wfc, bfc)
    res = run_bass_kernel_spmd(nc, in_maps, core_ids=list(range(NCORES)))
    out = np.empty((B, S, DIM), dtype=np.float32)
    for core in range(NCORES):
        b, h = core // 2, core % 2
        out[b, h * HALF:(h + 1) * HALF, :] = res.results[core]["y"]
    return out


# revision 30
# speedup vs baseline: 726.9120x; 2.0027x over previous
"""AttentionBlock kernel for 8 Trainium2 NeuronCores.

Problem (hardcoded): x [4, 2048, 1024] f32; Wq/Wk/Wv/Wfc [1024, 1024]; biases [1024].
    q = x@Wq.T+bq; k = x@Wk.T+bk; v = x@Wv.T+bv
    out = softmax(q k^T / sqrt(1024)) v;  y = out@Wfc.T+bfc + x

Sharding: core i = (b = i//2, h = i%2). Each core computes the full V / scores for
its batch element (duplicated across the 2 cores sharing a batch) and the
attention + fc for its half of the sequence. No collectives (measured ~40us fixed
+ ~7.6us/MB per 2-core AllGather here -- a K/V exchange costs more than it saves).

Key algebraic trick: q k^T = x (Wq^T Wk) x^T, so the host pre-contracts
M = Wq^T @ Wk and the kernel never materializes Q or K:
    G^T = M-blocks^T @ xT           (27us instead of Q-proj 27 + K-proj 55)
    S^T = xT-blocks^T @ G^T         (55us, lhsT streamed straight from x!)
The bias cross-terms are exact: the per-q term and constant cancel in softmax;
the per-k term r2[k] = x_k . (Wk^T bq) is a cheap rank-1 matmul folded into the
exp's per-partition bias.

Per-core plan (all matmuls float32r = full PE rate, ~2e-4 rel err):
  host feeds xT = x[b].T (d-major, rolled so this core's q-half is columns 0:1024)
  plus M, Wv^T, Wfc^T, so every GEMM has its contraction dim on partitions with
  no on-device transposes.
  - G^T [d, q] resident; V [s, e] resident (one xT sweep); r2 column per k-block
  - attention per q-chunk of 512: S^T blocks with xT streamed as lhsT, softmax
    over the partition (k) axis: exp(scale*S + r2) without max-subtract
    (|S|*scale <~ 6 here), denominator via ones-matmul, reciprocal broadcast
    across partitions with a rank-1 PE matmul, U^T = V-block.T @ expS^T
    accumulated in PSUM and normalized on copy-out -> O^T spilled to DRAM
  - fc: y = (O^T-block).T @ Wfc^T + bfc + x
"""

import numpy as np

B, S, DIM = 4, 2048, 1024
P = 128
NCORES = 8
HALF = S // 2          # 1024 q rows per core
DT = DIM // P          # 8 d tiles
ET = DIM // P          # 8 e tiles
SCH = S // 512         # 4 s-chunks for the V sweep
QC = 512               # attention q-chunk
NQ = HALF // QC        # 2 q chunks
KB = S // P            # 16 k blocks
SCALE = 1.0 / float(np.sqrt(DIM))

_CACHE = {}
TIMING_REPEAT = 21


def _build(repeat=1):
    import concourse.mybir as mybir
    import concourse.tile as tile
    from concourse import bacc

    F32 = mybir.dt.float32
    F32R = mybir.dt.float32r
    EXP = mybir.ActivationFunctionType.Exp
    IDENT = mybir.ActivationFunctionType.Identity
    ADD = mybir.AluOpType.add
    MULT = mybir.AluOpType.mult

    nc = bacc.Bacc()

    xt_d = nc.dram_tensor("xt", [DIM, S], F32R, kind="ExternalInput")
    xr_d = nc.dram_tensor("xr", [HALF, DIM], F32, kind="ExternalInput")
    m_d = nc.dram_tensor("m", [DIM, DIM], F32R, kind="ExternalInput")   # Wq^T Wk
    n_d = nc.dram_tensor("n", [DIM, DIM], F32R, kind="ExternalInput")   # Wv^T Wfc^T
    c2_d = nc.dram_tensor("c2", [DIM, 2], F32R, kind="ExternalInput")   # Wk^T bq, x2
    bvf_d = nc.dram_tensor("bvf", [DIM], F32, kind="ExternalInput")     # Wfc @ bv
    bf_d = nc.dram_tensor("bf", [DIM], F32, kind="ExternalInput")
    y_d = nc.dram_tensor("y", [HALF, DIM], F32, kind="ExternalOutput")

    xt3 = xt_d[:].rearrange("(dt p) s -> p dt s", p=P)      # [128, 8, 2048]
    m3 = m_d[:].rearrange("(dt p) e -> p dt e", p=P)
    n3 = n_d[:].rearrange("(dt p) e -> p dt e", p=P)
    c23 = c2_d[:].rearrange("(t p) w -> p t w", p=P)        # [128, 8, 2]

    with tile.TileContext(nc, pool_alloc_mode="stack") as tc:
        cpool = tc.alloc_tile_pool(name="const", bufs=1)
        ones2 = cpool.tile([P, 2], F32R)   # denominator rhs (even-N fp32r rule)
        ones_f32 = cpool.tile([P, P], F32)
        # Wk^T bq as columns per d-tile, duplicated x2 (fp32r matmuls need an
        # even moving free count)
        c2c = cpool.tile([P, DT, 2], F32R)
        nc.scalar.dma_start(c2c[:], c23)
        nc.vector.memset(ones_f32[:], 1.0)
        nc.vector.tensor_copy(ones2[:], ones_f32[:, 0:2])
        # warm the ACT LUTs (first use otherwise pays a ~1.4us cold table load)
        warm = cpool.tile([1, 2], F32)
        nc.scalar.activation(warm[0:1, 0:1], ones_f32[0:1, 0:1], IDENT)
        nc.scalar.activation(warm[0:1, 1:2], ones_f32[0:1, 0:1], EXP)
        # warm the PE HAM clock gate during the initial DMA wait: ~4us of dummy
        # matmuls with no input deps so the real work starts at 2.4GHz
        dwarm = cpool.tile([P, 512], F32R)
        nc.vector.memset(ones_f32[:], 1.0)
        nc.vector.tensor_copy(dwarm[:, 0:P], ones_f32[:])
        with tc.tile_pool(name="pwarm", bufs=1, space="PSUM") as pwp:
            pw = pwp.tile([2, 512], F32)
            for i in range(10):
                nc.tensor.matmul(pw[:], ones2[:], dwarm[:],
                                 start=(i == 0), stop=(i == 9))

        for _rep in range(repeat):
            # -------- Phase G: G^T = (Wq^T Wk)-blocks^T @ xT-half (resident) ----
            gpool = tc.alloc_tile_pool(name="gt", bufs=1)
            gt_sb = gpool.tile([P, DT, HALF], F32R, tag="gt")  # [d_p, d_tile, q]
            with tc.tile_pool(name="mq", bufs=1) as mqp, \
                 tc.tile_pool(name="xtq", bufs=2) as xtqp, \
                 tc.tile_pool(name="pq", bufs=3, space="PSUM") as pqp:
                m_sb = mqp.tile([P, DT, DIM], F32R)
                xtq0 = xtqp.tile([P, DT, 512], F32R, tag="xtq")
                # interleave the first loads across all three DMA queues so the
                # first group isn't gated by one queue's serial transfer rate
                engs = (nc.sync, nc.scalar, nc.gpsimd)
                for dt in range(DT):
                    engs[(2 * dt) % 3].dma_start(m_sb[:, dt, :], m3[:, dt, :])
                    engs[(2 * dt + 1) % 3].dma_start(xtq0[:, dt, :], xt3[:, dt, 0:512])
                for qch in range(HALF // 512):
                    if qch == 0:
                        xtq = xtq0
                    else:
                        xtq = xtqp.tile([P, DT, 512], F32R, tag="xtq")
                        nc.sync.dma_start(xtq[:], xt3[:, :, qch * 512:(qch + 1) * 512])
                    for dtile in range(DT):
                        ps = pqp.tile([P, 512], F32, tag="pq")
                        for dt in range(DT):
                            nc.tensor.matmul(
                                ps[:], m_sb[:, dt, dtile * P:(dtile + 1) * P],
                                xtq[:, dt, :],
                                start=(dt == 0), stop=(dt == DT - 1))
                        nc.scalar.activation(
                            gt_sb[:, dtile, qch * 512:(qch + 1) * 512], ps[:], IDENT)

            # ---- Phase VF: VF = x @ (Wv^T Wfc^T) + Wfc@bv -> SBUF resident ----
            # (P/denom) @ VF is then the fc output directly: the whole fc phase
            # and the O^T spill disappear. r2 columns computed in the same sweep.
            vpool = tc.alloc_tile_pool(name="vf", bufs=1)
            espool = tc.alloc_tile_pool(name="es", bufs=1)
            xtbpool = tc.alloc_tile_pool(name="xtb", bufs=3)
            vf_sb = vpool.tile([P, KB, DIM], F32R, tag="vf")  # [s_p, s_tile, e2]
            bvfb = vpool.tile([P, DIM], F32, tag="bvfb")
            r2c = vpool.tile([P, KB], F32, tag="r2c")  # scale*(x_k . Wk^T bq) per kb
            nc.scalar.dma_start(bvfb[:], bvf_d[:][None, :].to_broadcast((P, DIM)))
            with tc.tile_pool(name="wvp", bufs=1) as wvp, \
                 tc.tile_pool(name="xtk", bufs=2) as xtkp, \
                 tc.tile_pool(name="pkv", bufs=3, space="PSUM") as pkvp, \
                 tc.tile_pool(name="pr2", bufs=2, space="PSUM") as pr2p:
                n_sb = wvp.tile([P, DT, DIM], F32R, tag="n")
                xtk0 = xtkp.tile([P, DT, 256], F32R, tag="xtk")
                for dt in range(DT):
                    nc.sync.dma_start(n_sb[:, dt, :], n3[:, dt, :])
                    nc.gpsimd.dma_start(xtk0[:, dt, :], xt3[:, dt, 0:256])
                for sch in range(S // 256):
                    s0 = sch * 256
                    if sch == 0:
                        xtk = xtk0
                    else:
                        xtk = xtkp.tile([P, DT, 256], F32R, tag="xtk")
                        nc.sync.dma_start(xtk[:], xt3[:, :, s0:s0 + 256])
                    for st4 in range(2):
                        st = sch * 2 + st4
                        for eh in range(2):
                            ps = pkvp.tile([P, 512], F32, tag="pv")
                            for dt in range(DT):
                                nc.tensor.matmul(
                                    ps[:], xtk[:, dt, st4 * P:(st4 + 1) * P],
                                    n_sb[:, dt, eh * 512:(eh + 1) * 512],
                                    start=(dt == 0), stop=(dt == DT - 1))
                            nc.vector.tensor_tensor(
                                vf_sb[:, st, eh * 512:(eh + 1) * 512], ps[:],
                                bvfb[:, eh * 512:(eh + 1) * 512], ADD)
                        # r2 column for this k-block (exact bias cross-term)
                        pr = pr2p.tile([P, 2], F32, tag="pr2")
                        for dt in range(DT):
                            nc.tensor.matmul(
                                pr[:], xtk[:, dt, st4 * P:(st4 + 1) * P],
                                c2c[:, dt, :],
                                start=(dt == 0), stop=(dt == DT - 1))
                        nc.scalar.activation(r2c[:, st:st + 1], pr[:, 0:1], IDENT,
                                             scale=SCALE)

            # ------- Phase A: attention -> y directly (per q-chunk of 512) ------
            # S^T blocks -> exp -> per-q denominator columns; then
            # psum_y[q, e2] = sum_kb es-block^T @ VF-block gives the fc output in
            # natural layout (es is the stationary operand), normalized by a
            # per-partition 1/denom scale on the ACT copy-out, + bfc + x.
            bfbp = tc.alloc_tile_pool(name="bfbp", bufs=1)
            bfb = bfbp.tile([P, DIM], F32)
            nc.gpsimd.dma_start(bfb[:], bf_d[:][None, :].to_broadcast((P, DIM)))
            with tc.tile_pool(name="rec", bufs=2) as recp, \
                 tc.tile_pool(name="xrt", bufs=3) as xrp, \
                 tc.tile_pool(name="ysb", bufs=4) as ysp, \
                 tc.tile_pool(name="ps_s", bufs=2, space="PSUM") as psp, \
                 tc.tile_pool(name="ps_y", bufs=3, space="PSUM") as pyp, \
                 tc.tile_pool(name="ps_d", bufs=2, space="PSUM") as pdp:
                for qc in range(NQ):
                    q0 = qc * QC
                    es = espool.tile([P, KB, QC], F32R, tag="es")  # exp [k_p, kb, q]
                    for kb in range(KB):
                        xtb = xtbpool.tile([P, DT, P], F32R, tag="xtb")
                        nc.sync.dma_start(xtb[:], xt3[:, :, kb * P:(kb + 1) * P])
                        ps = psp.tile([P, QC], F32, tag="ps_s")
                        for dt in range(DT):
                            nc.tensor.matmul(
                                ps[:], xtb[:, dt, :], gt_sb[:, dt, q0:q0 + QC],
                                start=(dt == 0), stop=(dt == DT - 1))
                        nc.scalar.activation(es[:, kb, :], ps[:], EXP,
                                             bias=r2c[:, kb:kb + 1], scale=SCALE)
                    # per-q denominator columns + reciprocal, one per q-block
                    recq = recp.tile([P, QC // P], F32, tag="recq")
                    for qb in range(QC // P):
                        pd = pdp.tile([P, 2], F32, tag="ps_d")
                        for kb in range(KB):
                            nc.tensor.matmul(
                                pd[:], es[:, kb, qb * P:(qb + 1) * P], ones2[:],
                                start=(kb == 0), stop=(kb == KB - 1))
                        with nc.allow_low_precision(reason="per-partition scale vec"):
                            nc.vector.reciprocal(recq[:, qb:qb + 1], pd[:, 0:1])
                    # y = (es/denom)^T @ VF + bfc + x, written straight out
                    for qb in range(QC // P):
                        q_t = qc * (QC // P) + qb
                        xrt = xrp.tile([P, DIM], F32, tag="xrt")
                        nc.scalar.dma_start(xrt[:], xr_d[q_t * P:(q_t + 1) * P, :])
                        for ec in range(2):
                            py = pyp.tile([P, 512], F32, tag="ps_y")
                            for kb in range(KB):
                                nc.tensor.matmul(
                                    py[:], es[:, kb, qb * P:(qb + 1) * P],
                                    vf_sb[:, kb, ec * 512:(ec + 1) * 512],
                                    start=(kb == 0), stop=(kb == KB - 1))
                            ysb = ysp.tile([P, 512], F32, tag="ysb")
                            nc.scalar.activation(ysb[:], py[:], IDENT,
                                                 scale=recq[:, qb:qb + 1])
                            nc.vector.tensor_tensor(
                                ysb[:], ysb[:], bfb[:, ec * 512:(ec + 1) * 512], ADD)
                            nc.vector.tensor_tensor(
                                ysb[:], ysb[:], xrt[:, ec * 512:(ec + 1) * 512], ADD)
                            nc.gpsimd.dma_start(
                                y_d[q_t * P:(q_t + 1) * P, ec * 512:(ec + 1) * 512],
                                ysb[:])

            bfbp.release()
            xtbpool.release()
            espool.release()
            vpool.release()
            gpool.release()
        cpool.release()

    nc.finalize()
    return nc


def _get_nc():
    if "nc" not in _CACHE:
        _CACHE["nc"] = _build()
    return _CACHE["nc"]


def _make_in_maps(x, Wq, bq, Wk, bk, Wv, bv, Wfc, bfc):
    x = np.asarray(x, dtype=np.float32)
    Wq = np.asarray(Wq, np.float32); Wk = np.asarray(Wk, np.float32)
    Wv = np.asarray(Wv, np.float32); Wfc = np.asarray(Wfc, np.float32)
    m = np.ascontiguousarray(Wq.T @ Wk)            # q k^T = x m x^T
    n = np.ascontiguousarray(Wv.T @ Wfc.T)         # (P/denom) @ (x n) = fc out
    c2v = Wk.T @ np.asarray(bq, np.float32)
    c2 = np.ascontiguousarray(np.repeat(c2v[:, None], 2, axis=1))
    bvf = np.ascontiguousarray(Wfc @ np.asarray(bv, np.float32))
    bf = np.asarray(bfc, np.float32)

    in_maps = []
    for core in range(NCORES):
        b, h = core // 2, core % 2
        xtb = np.ascontiguousarray(x[b].T)  # [DIM, S]
        # roll so this core's q-half sits at columns [0, HALF); the k ordering
        # permutes consistently in scores and V, and softmax+sum over k is
        # permutation-invariant, so one SPMD program serves both halves.
        xt = np.ascontiguousarray(np.roll(xtb, -h * HALF, axis=1)) if h else xtb
        in_maps.append({
            "xt": xt,
            "xr": np.ascontiguousarray(x[b, h * HALF:(h + 1) * HALF, :]),
            "m": m, "n": n, "c2": c2, "bvf": bvf, "bf": bf,
        })
    return in_maps


def kernel(x, Wq, bq, Wk, bk, Wv, bv, Wfc, bfc):
    from concourse.bass_utils import run_bass_kernel_spmd

    nc = _get_nc()
    in_maps = _make_in_maps(x, Wq, bq, Wk, bk, Wv, bv, Wfc, bfc)
    res = run_bass_kernel_spmd(nc, in_maps, core_ids=list(range(NCORES)))
    out = np.empty((B, S, DIM), dtype=np.float32)
    for core in range(NCORES):
        b, h = core // 2, core % 2
        out[b, h * HALF:(h + 1) * HALF, :] = res.results[core]["y"]
    return out
